# revision 5
# baseline (speedup 1.0000x reference)
"""Additive (Bahdanau) attention log-softmax weights on 8 TRN2 NeuronCores.

Math (per batch b, head 0):
    qp = Q @ Wq^T ; kp = K @ Wk^T          (Wc = [Wq | Wk], both [D, D])
    logit[q, k] = Wl . tanh(qp[q] + kp[k] + bc) + bl + where(mask[k]==0, -1e9, 1.0)
    out[q, :]   = log_softmax(logit[q, :])

Distribution: pure data parallel, core c <- (batch b = c//2, q-half c%2),
no collectives.  Sparse-attention trick: keys with mask==0 only need
out = -1e9 - LSE (error O(1) vs magnitude 1e9), so the device only computes
the ~136 valid keys (host compacts + pads to V).

Algorithm: polynomial separation instead of elementwise tanh.  With
A[e,q] = qp^T and K[e,k] = kp^T + bc, fit an odd minimax polynomial
p(x) = sum_j c_j x^j ~ tanh(x) on the exact realized range of A+K, then

    logit[q,k] ~ sum_e Wl[e] p(A+K) = sum_m <(Wl .* A^m)[:,q], R_m(K)[:,k]>

where R_m(K) = sum_l c_{m+l} C(m+l,m) K^l is elementwise in K.  The m=n
term is constant in k and cancels in log_softmax -> dropped.  This turns
8.9M tanh evaluations per core (58us on ACT at 1 col/cycle) into
 - a rank-(n*512) PE contraction (36 matmuls, ~4us),
 - Horner chains for R_m on DVE over only [512, V] elements (~12us),
 - powers A^m on DVE over [512, 128] (~3us),
and the logits land dense in PSUM [q, V], so softmax needs no gather.

Per-core layout (e' = partition, 4 e-chunks "ec" in the free dim):
  A, P_m : [128, 512]  col = ec*128 + q      (bf16)
  K, K2, R_m : [128, 4V]  col = ec*V + kc    (bf16)
  psL    : PSUM [128q, V] accumulating all 36 (m, ec) matmuls.
R_m chains are emitted step-round-robin (m descending) so R_8 finishes
first and PE starts accumulating while R_0 is still being built.

Timing note: the repeat-slope NEFF shares ONE output DRAM tensor across
repeats -- the axon tunnel charges ~80ms per output tensor (independent
of size/compute), which would otherwise dominate the slope.
"""

import numpy as np
import ml_dtypes
from contextlib import ExitStack

import concourse.bass as bass
import concourse.tile as tile
from concourse import bacc, mybir
from concourse.bass_utils import run_bass_kernel_spmd

F32 = mybir.dt.float32
BF16 = mybir.dt.bfloat16
AF = mybir.ActivationFunctionType
ALU = mybir.AluOpType

B, H, Lq, Lkv, D = 4, 1, 256, 256, 512
NCORES = 8
LQL = Lq // 2          # q rows per core
NEG = -1.0e9
NPOLY = 9              # odd minimax degree for tanh

_nc_cache: dict[int, object] = {}


def _chain_specs(n=NPOLY):
    """Per-m Horner chain structure for R_m(K) = sum_l beta_l K^l with
    m+l odd, l <= n-m.  Returns [(m, kfactor, d2)] with d2 = degree in K2
    of the inner polynomial; m even -> R_m = K * poly_d2(K2) (no constant),
    m odd -> R_m = poly_d2(K2) (with constant).  m = n dropped (k-constant
    term cancels in log_softmax)."""
    specs = []
    for m in range(n):
        lmax = n - m if (n - m + m) % 2 == 1 else n - m - 1
        # l of the same parity as (odd - m): l parity = (1 - m%2)
        if m % 2 == 0:
            d2 = (lmax - 1) // 2      # l = 1, 3, ..., lmax = 2*d2+1
            specs.append((m, True, d2))
        else:
            d2 = lmax // 2            # l = 0, 2, ..., lmax = 2*d2
            specs.append((m, False, d2))
    return specs


def _chain_coeffs(cj, n=NPOLY):
    """Flat coefficient list in the exact order _build consumes them.
    For chain m the Horner (over y=K2) coefficients are beta_{l(top)} ...
    beta_{l(bottom)}, where beta_l = c_{m+l} * C(m+l, m)."""
    from math import comb
    out = []
    for m, kfac, d2 in _chain_specs(n):
        if kfac:
            ls = [2 * i + 1 for i in range(d2, -1, -1)]
        else:
            ls = [2 * i for i in range(d2, -1, -1)]
        for l in ls:
            out.append(float(cj[m + l]) * comb(m + l, m))
    return out


def _build(V: int, repeats: int = 1):
    """Build + schedule the per-core Bass graph for padded-valid-count V."""
    nc = bacc.Bacc(None, target_bir_lowering=False)

    specs = _chain_specs()
    ncoef = sum(d2 + 1 for _, _, d2 in specs)

    p_qt = nc.declare_dram_parameter("qt", [128, 512], BF16, isOutput=False)
    p_kt = nc.declare_dram_parameter("kt", [128, 4 * V], BF16, isOutput=False)
    p_wct = nc.declare_dram_parameter("wct", [4, 128, 1024], BF16, isOutput=False)
    p_bcp = nc.declare_dram_parameter("bcp", [128, 4], F32, isOutput=False)
    p_wlp = nc.declare_dram_parameter("wlp", [128, 4], F32, isOutput=False)
    p_p0 = nc.declare_dram_parameter("p0", [128, 512], BF16, isOutput=False)
    p_coef = nc.declare_dram_parameter("coef", [128, ncoef], F32, isOutput=False)
    p_bv = nc.declare_dram_parameter("bv", [128, V], F32, isOutput=False)
    # One output tensor shared by every repeat: the axon tunnel charges a
    # large fixed cost PER OUTPUT TENSOR, so the timing NEFF must not scale
    # its output count with R (WAW between repeats is queue-ordered).
    p_out = nc.declare_dram_parameter("out", [128, V + 1], F32, isOutput=True)

    with ExitStack() as ctx:
        tc = ctx.enter_context(tile.TileContext(nc))
        const = ctx.enter_context(tc.tile_pool(name="const", bufs=1))
        psum = ctx.enter_context(tc.tile_pool(name="psum", bufs=2, space="PSUM"))

        for rep in range(repeats):
            r = f"_r{rep}"
            # ---- loads ----
            wct_t = [const.tile([128, 1024], BF16, tag=f"wct{ec}",
                                name=f"wct{ec}{r}") for ec in range(4)]
            nc.sync.dma_start(wct_t[0][:], p_wct[0])
            qt_t = const.tile([128, 512], BF16, tag="qt", name=f"qt{r}")
            nc.sync.dma_start(qt_t[:], p_qt[:])
            kt_t = const.tile([128, 4 * V], BF16, tag="kt", name=f"kt{r}")
            nc.sync.dma_start(kt_t[:], p_kt[:])
            bcp_t = const.tile([128, 4], F32, tag="bcp", name=f"bcp{r}")
            nc.sync.dma_start(bcp_t[:], p_bcp[:])
            wlp_t = const.tile([128, 4], F32, tag="wlp", name=f"wlp{r}")
            nc.sync.dma_start(wlp_t[:], p_wlp[:])
            coef_t = const.tile([128, ncoef], F32, tag="coef", name=f"coef{r}")
            nc.sync.dma_start(coef_t[:], p_coef[:])
            p0_t = const.tile([128, 512], BF16, tag="p0", name=f"p0{r}")
            nc.sync.dma_start(p0_t[:], p_p0[:])
            bv_t = const.tile([128, V], F32, tag="bv", name=f"bv{r}")
            nc.sync.dma_start(bv_t[:], p_bv[:])
            for ec in range(1, 4):
                nc.sync.dma_start(wct_t[ec][:], p_wct[ec])

            # ---- projections: qp^T -> A/P1, kp^T + bc -> K ----
            K = const.tile([128, 4 * V], BF16, tag="K", name=f"K{r}")
            K2 = const.tile([128, 4 * V], BF16, tag="K2", name=f"K2{r}")
            A = const.tile([128, 512], BF16, tag="A", name=f"A{r}")
            P = [p0_t] + [const.tile([128, 512], BF16, tag=f"P{m}",
                                     name=f"P{m}{r}") for m in range(1, NPOLY)]
            R = [const.tile([128, 4 * V], BF16, tag=f"R{m}", name=f"R{m}{r}")
                 for m in range(NPOLY)]
            for ec in range(4):
                ps_k = psum.tile([128, V], F32, tag="psk", name=f"psk{ec}{r}")
                for dc in range(4):
                    nc.tensor.matmul(
                        ps_k[:],
                        wct_t[ec][:, 512 + dc * 128:512 + (dc + 1) * 128],
                        kt_t[:, dc * V:(dc + 1) * V],
                        start=(dc == 0), stop=(dc == 3))
                nc.vector.tensor_scalar_add(
                    K[:, ec * V:(ec + 1) * V], ps_k[:], bcp_t[:, ec:ec + 1])
                ps_q = psum.tile([128, 128], F32, tag="psq", name=f"psq{ec}{r}")
                for dc in range(4):
                    nc.tensor.matmul(
                        ps_q[:], wct_t[ec][:, dc * 128:(dc + 1) * 128],
                        qt_t[:, dc * 128:(dc + 1) * 128],
                        start=(dc == 0), stop=(dc == 3))
                nc.vector.tensor_copy(A[:, ec * 128:(ec + 1) * 128], ps_q[:])
                nc.vector.tensor_scalar_mul(
                    P[1][:, ec * 128:(ec + 1) * 128], ps_q[:],
                    wlp_t[:, ec:ec + 1])

            # ---- elementwise prep on DVE ----
            nc.vector.tensor_tensor(K2[:], K[:], K[:], ALU.mult)
            for m in range(2, NPOLY):
                nc.vector.tensor_tensor(P[m][:], P[m - 1][:], A[:], ALU.mult)

            # R_m Horner chains over K2, emitted step-round-robin with m
            # descending so the shortest chain (largest m) completes first
            # and PE can start accumulating early.  Coefficient column
            # indices follow _chain_coeffs order.
            cbase = {}
            idx = 0
            for m, kfac, d2 in specs:
                cbase[m] = idx
                idx += d2 + 1
            # step 0: acc = beta_top * K2  (chains with d2 >= 1)
            #         (for d2 == 0 chains: acc = beta_top broadcast handled
            #          in the finish step below)
            order = [s for s in reversed(specs)]   # m = 8 .. 0
            # chain state: number of emitted Horner mults
            maxsteps = max(d2 for _, _, d2 in specs) + 2
            done = set()
            for step in range(maxsteps + 1):
                for m, kfac, d2 in order:
                    if m in done:
                        continue
                    acc = R[m]
                    c0 = cbase[m]
                    if d2 == 0:
                        # single coefficient: R = beta * K  (odd n makes a
                        # constant-only chain impossible for m < n)
                        assert kfac
                        nc.vector.tensor_scalar_mul(
                            acc[:], K[:], coef_t[:, c0:c0 + 1])
                        done.add(m)
                        continue
                    if step == 0:
                        nc.vector.tensor_scalar_mul(
                            acc[:], K2[:], coef_t[:, c0:c0 + 1])
                        nc.vector.tensor_scalar_add(
                            acc[:], acc[:], coef_t[:, c0 + 1:c0 + 2])
                    elif step < d2:
                        nc.vector.tensor_tensor(acc[:], acc[:], K2[:], ALU.mult)
                        nc.vector.tensor_scalar_add(
                            acc[:], acc[:], coef_t[:, c0 + step + 1:c0 + step + 2])
                    elif step == d2:
                        if kfac:
                            nc.vector.tensor_tensor(acc[:], acc[:], K[:], ALU.mult)
                        done.add(m)
                    # step > d2 shouldn't happen once done
            # ---- PE: logits = sum_m <P_m, R_m>, m descending ----
            psL = psum.tile([128, V], F32, tag="psL", name=f"psL{r}")
            mm = [(m, ec) for m in range(NPOLY - 1, -1, -1) for ec in range(4)]
            for i, (m, ec) in enumerate(mm):
                nc.tensor.matmul(
                    psL[:], P[m][:, ec * 128:(ec + 1) * 128],
                    R[m][:, ec * V:(ec + 1) * V],
                    start=(i == 0), stop=(i == len(mm) - 1))

            # ---- softmax tail ----
            lgb = const.tile([128, V], F32, tag="lgb", name=f"lgb{r}")
            ex = const.tile([128, V], F32, tag="ex", name=f"ex{r}")
            sm = const.tile([128, 1], F32, tag="sm", name=f"sm{r}")
            lsm = const.tile([128, 1], F32, tag="lsm", name=f"lsm{r}")
            ov = const.tile([128, V + 1], F32, tag="ov", name=f"ov{r}")
            nc.vector.tensor_tensor(lgb[:], psL[:], bv_t[:], ALU.add)
            nc.scalar.activation(ex[:], lgb[:], AF.Exp, accum_out=sm[:])
            nc.scalar.activation(lsm[:], sm[:], AF.Ln)
            nc.vector.tensor_scalar_sub(ov[:, 0:V], lgb[:], lsm[:, 0:1])
            nc.vector.tensor_scalar(
                ov[:, V:V + 1], lsm[:], -1.0, NEG,
                op0=ALU.mult, op1=ALU.add)
            nc.sync.dma_start(p_out[:], ov[:])

    nc.compile()
    return nc


def _fit_poly(X, n=NPOLY):
    """Least-squares odd-poly fit of tanh on [-X, X] at Chebyshev nodes."""
    x = X * np.cos(np.linspace(0, np.pi, 4001))
    pows = np.arange(1, n + 1, 2)
    M = x[:, None] ** pows[None, :]
    c, *_ = np.linalg.lstsq(M, np.tanh(x), rcond=None)
    cj = np.zeros(n + 1)
    cj[pows] = c
    return cj


def _prep(queries, keys, values, mask, Wc, bc, Wl, bl):
    """Host-side sharding: returns (V, in_maps, idx_valid, idx_masked)."""
    mask = np.asarray(mask)
    idx_v = [np.nonzero(mask[b])[0] for b in range(B)]
    idx_m = [np.nonzero(mask[b] == 0)[0] for b in range(B)]
    maxv = max(len(ix) for ix in idx_v)
    V = max(136, -(-maxv // 8) * 8)

    bf = ml_dtypes.bfloat16
    q_np = np.asarray(queries, np.float32)
    k_np = np.asarray(keys, np.float32)
    Wc_np = np.asarray(Wc, np.float32)
    bc_np = np.asarray(bc, np.float32)
    Wl_np = np.asarray(Wl, np.float32)[0]
    blv = float(np.asarray(bl, np.float32)[0])

    # poly fit on the exact realized range of A + K (computed on host; the
    # projections are cheap in fp32 BLAS and only run on the correctness path)
    qp = np.einsum('bhqd,ed->bqe', q_np, Wc_np[:, :D], optimize=True)
    kp = np.einsum('bhkd,ed->bke', k_np, Wc_np[:, D:], optimize=True) + bc_np
    xmax = max(float((qp[b].max(0) + kp[b].max(0)).max()) for b in range(B))
    xmin = min(float((qp[b].min(0) + kp[b].min(0)).min()) for b in range(B))
    X = max(abs(xmax), abs(xmin)) * 1.02
    cj = _fit_poly(X)
    coefs = _chain_coeffs(cj)
    coef = np.tile(np.asarray(coefs, np.float32), (128, 1))

    wct_full = Wc_np.T.astype(bf)       # [2D, D]
    wct = np.empty((4, 128, 1024), bf)
    for ec in range(4):
        for dc in range(4):
            wct[ec, :, dc * 128:(dc + 1) * 128] = \
                wct_full[dc * 128:(dc + 1) * 128, ec * 128:(ec + 1) * 128]
            wct[ec, :, 512 + dc * 128:512 + (dc + 1) * 128] = \
                wct_full[D + dc * 128:D + (dc + 1) * 128, ec * 128:(ec + 1) * 128]
    bcp = np.ascontiguousarray(bc_np.reshape(4, 128).T)
    wlp = np.ascontiguousarray(Wl_np.reshape(4, 128).T)
    p0 = np.ascontiguousarray(
        np.repeat(wlp.T.astype(bf)[:, :, None], 128, axis=2)
        .transpose(1, 0, 2).reshape(128, 512))

    in_maps = []
    for c in range(NCORES):
        b, qh = c // 2, c % 2
        qt_d = q_np[b, 0, qh * LQL:(qh + 1) * LQL, :].T.astype(bf)   # [D, LQL]
        qt = np.ascontiguousarray(
            qt_d.reshape(4, 128, LQL).transpose(1, 0, 2).reshape(128, 512))
        ktc = np.zeros((D, V), bf)
        ktc[:, :len(idx_v[b])] = k_np[b, 0, idx_v[b], :].T.astype(bf)
        kt = np.ascontiguousarray(
            ktc.reshape(4, 128, V).transpose(1, 0, 2).reshape(128, 4 * V))
        bvrow = np.full(V, NEG, np.float32)
        bvrow[:len(idx_v[b])] = 1.0 + blv
        bv = np.tile(bvrow, (128, 1))
        in_maps.append({
            "qt": qt, "kt": kt, "wct": wct, "bcp": bcp, "wlp": wlp,
            "p0": p0, "coef": coef, "bv": np.ascontiguousarray(bv),
        })
    return V, in_maps, idx_v, idx_m


def kernel(queries, keys, values, mask, Wc, bc, Wl, bl):
    V, in_maps, idx_v, idx_m = _prep(queries, keys, values, mask, Wc, bc, Wl, bl)
    if V not in _nc_cache:
        _nc_cache[V] = _build(V)
    nc = _nc_cache[V]
    res = run_bass_kernel_spmd(nc, in_maps, core_ids=list(range(NCORES))).results

    full = np.empty((B, Lq, Lkv), np.float32)
    for c in range(NCORES):
        b, qh = c // 2, c % 2
        o = np.asarray(res[c]["out"], np.float32)      # [128, V+1]
        nv = len(idx_v[b])
        blk = full[b, qh * LQL:(qh + 1) * LQL]          # [128, Lkv]
        blk[:, idx_v[b]] = o[:, :nv]
        blk[:, idx_m[b]] = o[:, V:V + 1]
    return full


# revision 13
# speedup vs baseline: 37.6205x; 37.6205x over previous
"""Additive (Bahdanau) attention log-softmax weights on 8 TRN2 NeuronCores.

Math (per batch b, head 0):
    qp = Q @ Wq^T ; kp = K @ Wk^T          (Wc = [Wq | Wk], both [D, D])
    logit[q, k] = Wl . tanh(qp[q] + kp[k] + bc) + bl + where(mask[k]==0, -1e9, 1.0)
    out[q, :]   = log_softmax(logit[q, :])

Distribution: pure data parallel, core c <- (batch b = c//2, q-half c%2),
no collectives.  Sparse-attention trick: keys with mask==0 only need
out = -1e9 - LSE (error O(1) vs magnitude 1e9), so the device only computes
the ~136 valid keys (host compacts + pads to V).

Algorithm: polynomial separation instead of elementwise tanh.  With
A[e,q] = qp^T and K[e,k] = kp^T + bc, fit an odd minimax polynomial
p(x) = sum_j c_j x^j ~ tanh(x) on the exact realized range of A+K, then

    logit[q,k] ~ sum_e Wl[e] p(A+K) = sum_m <(Wl .* A^m)[:,q], R_m(K)[:,k]>

where R_m(K) = sum_l c_{m+l} C(m+l,m) K^l is elementwise in K.  The m=n
term is constant in k and cancels in log_softmax -> dropped.  This turns
8.9M tanh evaluations per core (58us on ACT at 1 col/cycle) into
 - a rank-(n*512) PE contraction (36 matmuls, ~4us),
 - Horner chains for R_m on DVE over only [512, V] elements (~12us),
 - powers A^m on DVE over [512, 128] (~3us),
and the logits land dense in PSUM [q, V], so softmax needs no gather.

Per-core layout (e' = partition, 4 e-chunks "ec" in the free dim):
  A, P_m : [128, 512]  col = ec*128 + q      (bf16)
  K, K2, R_m : [128, 4V]  col = ec*V + kc    (bf16)
  psL    : PSUM [128q, V] accumulating all 36 (m, ec) matmuls.
R_m chains are emitted step-round-robin (m descending) so R_8 finishes
first and PE starts accumulating while R_0 is still being built.

Timing note: the repeat-slope NEFF shares ONE output DRAM tensor across
repeats -- the axon tunnel charges ~80ms per output tensor (independent
of size/compute), which would otherwise dominate the slope.
"""

import numpy as np
import ml_dtypes
from contextlib import ExitStack

import concourse.bass as bass
import concourse.tile as tile
from concourse import bacc, mybir
from concourse.bass_utils import run_bass_kernel_spmd

F32 = mybir.dt.float32
BF16 = mybir.dt.bfloat16
AF = mybir.ActivationFunctionType
ALU = mybir.AluOpType

B, H, Lq, Lkv, D = 4, 1, 256, 256, 512
NCORES = 8
LQL = Lq // 2          # q rows per core
NEG = -1.0e9
NPOLY = 9              # odd minimax degree for tanh

_nc_cache: dict[int, object] = {}


def _chain_specs(n=NPOLY):
    """Per-m Horner chain structure for R_m(K) = sum_l beta_l K^l with
    m+l odd, l <= n-m.  Returns [(m, kfactor, d2)] with d2 = degree in K2
    of the inner polynomial; m even -> R_m = K * poly_d2(K2) (no constant),
    m odd -> R_m = poly_d2(K2) (with constant).  m = n dropped (k-constant
    term cancels in log_softmax)."""
    specs = []
    for m in range(n):
        lmax = n - m if (n - m + m) % 2 == 1 else n - m - 1
        # l of the same parity as (odd - m): l parity = (1 - m%2)
        if m % 2 == 0:
            d2 = (lmax - 1) // 2      # l = 1, 3, ..., lmax = 2*d2+1
            specs.append((m, True, d2))
        else:
            d2 = lmax // 2            # l = 0, 2, ..., lmax = 2*d2
            specs.append((m, False, d2))
    return specs


def _chain_coeffs(cj, n=NPOLY):
    """Flat coefficient list in the exact order _build consumes them.
    For chain m the Horner (over y=K2) coefficients are beta_{l(top)} ...
    beta_{l(bottom)}, where beta_l = c_{m+l} * C(m+l, m)."""
    from math import comb
    out = []
    for m, kfac, d2 in _chain_specs(n):
        if kfac:
            ls = [2 * i + 1 for i in range(d2, -1, -1)]
        else:
            ls = [2 * i for i in range(d2, -1, -1)]
        for l in ls:
            out.append(float(cj[m + l]) * comb(m + l, m))
    return out


def _build(V: int, repeats: int = 1):
    """Build + schedule the per-core Bass graph for padded-valid-count V."""
    nc = bacc.Bacc(None, target_bir_lowering=False)

    specs = _chain_specs()
    ncoef = sum(d2 + 1 for _, _, d2 in specs)

    # packed inputs: qkp = [qt | p0 | kt] bf16, aux = [bcp | wlp | coef | bv]
    # f32 -- each dma_start costs ~0.6us of sequencer dispatch, so small
    # tensors ride together.  wct loads go on the ACT queue so they overlap
    # the qkp/aux loads on the SP queue.
    p_qkp = nc.declare_dram_parameter("qkp", [128, 1024 + 4 * V], BF16,
                                      isOutput=False)
    p_wct = nc.declare_dram_parameter("wct", [4, 128, 1024], BF16, isOutput=False)
    p_aux = nc.declare_dram_parameter("aux", [128, 8 + ncoef + V], F32,
                                      isOutput=False)
    # One output tensor shared by every repeat: the axon tunnel charges a
    # large fixed cost PER OUTPUT TENSOR, so the timing NEFF must not scale
    # its output count with R (WAW between repeats is queue-ordered).
    p_out = nc.declare_dram_parameter("out", [128, V + 1], F32, isOutput=True)

    with ExitStack() as ctx:
        tc = ctx.enter_context(tile.TileContext(nc))
        const = ctx.enter_context(tc.tile_pool(name="const", bufs=1))
        psum = ctx.enter_context(tc.tile_pool(name="psum", bufs=2, space="PSUM"))

        for rep in range(repeats):
            r = f"_r{rep}"
            # ---- loads (wct on the ACT queue, rest on SP queue) ----
            wct_t = [const.tile([128, 1024], BF16, tag=f"wct{ec}",
                                name=f"wct{ec}{r}") for ec in range(4)]
            for ec in range(4):
                nc.gpsimd.dma_start(wct_t[ec][:], p_wct[ec])
            qkp_t = const.tile([128, 1024 + 4 * V], BF16, tag="qkp",
                               name=f"qkp{r}")
            nc.sync.dma_start(qkp_t[:], p_qkp[:])
            aux_t = const.tile([128, 8 + ncoef + V], F32, tag="aux",
                               name=f"aux{r}")
            nc.sync.dma_start(aux_t[:], p_aux[:])
            qt_t = qkp_t[:, 0:512]
            p0_t = qkp_t[:, 512:1024]
            kt_t = qkp_t[:, 1024:1024 + 4 * V]
            bcp_t = aux_t[:, 0:4]
            wlp_t = aux_t[:, 4:8]
            coef_t = aux_t[:, 8:8 + ncoef]
            bv_t = aux_t[:, 8 + ncoef:8 + ncoef + V]

            # ---- projections: qp^T -> A/P1, kp^T + bc -> K (folds on the
            #      otherwise-idle ACT engine; per-partition bias/scale APs) --
            K = const.tile([128, 4 * V], BF16, tag="K", name=f"K{r}")
            K2 = const.tile([128, 4 * V], BF16, tag="K2", name=f"K2{r}")
            A = const.tile([128, 512], BF16, tag="A", name=f"A{r}")
            P = [p0_t] + [const.tile([128, 512], BF16, tag=f"P{m}",
                                     name=f"P{m}{r}") for m in range(1, NPOLY)]
            R = [const.tile([128, 4 * V], BF16, tag=f"R{m}", name=f"R{m}{r}")
                 for m in range(NPOLY)]
            # k-projections first: the K -> K2 -> R chains are the critical
            # path; q-projections only feed the P side which has slack
            for ec in range(4):
                ps_k = psum.tile([128, V], F32, tag="psk", name=f"psk{ec}{r}")
                for dc in range(4):
                    nc.tensor.matmul(
                        ps_k[:],
                        wct_t[ec][:, 512 + dc * 128:512 + (dc + 1) * 128],
                        kt_t[:, dc * V:(dc + 1) * V],
                        start=(dc == 0), stop=(dc == 3))
                nc.scalar.activation(
                    K[:, ec * V:(ec + 1) * V], ps_k[:], AF.Identity,
                    bias=bcp_t[:, ec:ec + 1])
            nc.vector.tensor_tensor(K2[:], K[:], K[:], ALU.mult)
            for ec in range(4):
                ps_q = psum.tile([128, 128], F32, tag="psq", name=f"psq{ec}{r}")
                for dc in range(4):
                    nc.tensor.matmul(
                        ps_q[:], wct_t[ec][:, dc * 128:(dc + 1) * 128],
                        qt_t[:, dc * 128:(dc + 1) * 128],
                        start=(dc == 0), stop=(dc == 3))
                nc.scalar.activation(
                    A[:, ec * 128:(ec + 1) * 128], ps_q[:], AF.Copy)
                nc.scalar.activation(
                    P[1][:, ec * 128:(ec + 1) * 128], ps_q[:], AF.Copy,
                    scale=wlp_t[:, ec:ec + 1])

            # ---- P-power chain on the (otherwise idle) Pool engine: its
            #      latency is off the critical path, which runs through the
            #      R chains on DVE ----
            for m in range(2, NPOLY):
                nc.gpsimd.tensor_tensor(P[m][:], P[m - 1][:], A[:], ALU.mult)

            # R_m Horner chains over K2, emitted step-round-robin with m
            # descending so the shortest chain (largest m) completes first
            # and PE can start accumulating early.  Coefficient column
            # indices follow _chain_coeffs order.
            cbase = {}
            idx = 0
            for m, kfac, d2 in specs:
                cbase[m] = idx
                idx += d2 + 1
            # step 0: acc = beta_top * K2  (chains with d2 >= 1)
            #         (for d2 == 0 chains: acc = beta_top broadcast handled
            #          in the finish step below)
            order = [s for s in reversed(specs)]   # m = 8 .. 0
            # chain state: number of emitted Horner mults
            maxsteps = max(d2 for _, _, d2 in specs) + 2
            done = set()
            for step in range(maxsteps + 1):
                for m, kfac, d2 in order:
                    if m in done:
                        continue
                    acc = R[m]
                    c0 = cbase[m]
                    if d2 == 0:
                        # single coefficient: R = beta * K  (odd n makes a
                        # constant-only chain impossible for m < n)
                        assert kfac
                        nc.vector.tensor_scalar_mul(
                            acc[:], K[:], coef_t[:, c0:c0 + 1])
                        done.add(m)
                        continue
                    if step == 0:
                        # fused (K2 * beta_top + beta_next) in one 4x instr
                        nc.vector.tensor_scalar(
                            acc[:], K2[:], coef_t[:, c0:c0 + 1],
                            coef_t[:, c0 + 1:c0 + 2],
                            op0=ALU.mult, op1=ALU.add)
                    elif step < d2:
                        nc.vector.tensor_tensor(acc[:], acc[:], K2[:], ALU.mult)
                        nc.vector.tensor_scalar_add(
                            acc[:], acc[:], coef_t[:, c0 + step + 1:c0 + step + 2])
                    elif step == d2:
                        if kfac:
                            nc.vector.tensor_tensor(acc[:], acc[:], K[:], ALU.mult)
                        done.add(m)
                    # step > d2 shouldn't happen once done
            # ---- PE: logits = sum_m <P_m, R_m>, m descending ----
            psL = psum.tile([128, V], F32, tag="psL", name=f"psL{r}")
            mm = [(m, ec) for m in range(NPOLY - 1, -1, -1) for ec in range(4)]
            for i, (m, ec) in enumerate(mm):
                nc.tensor.matmul(
                    psL[:], P[m][:, ec * 128:(ec + 1) * 128],
                    R[m][:, ec * V:(ec + 1) * V],
                    start=(i == 0), stop=(i == len(mm) - 1))

            # ---- softmax tail ----
            lgb = const.tile([128, V], F32, tag="lgb", name=f"lgb{r}")
            ex = const.tile([128, V], F32, tag="ex", name=f"ex{r}")
            sm = const.tile([128, 1], F32, tag="sm", name=f"sm{r}")
            lsm = const.tile([128, 1], F32, tag="lsm", name=f"lsm{r}")
            ov = const.tile([128, V + 1], F32, tag="ov", name=f"ov{r}")
            nc.vector.tensor_tensor(lgb[:], psL[:], bv_t[:], ALU.add)
            nc.scalar.activation(ex[:], lgb[:], AF.Exp, accum_out=sm[:])
            nc.scalar.activation(lsm[:], sm[:], AF.Ln)
            nc.vector.tensor_scalar_sub(ov[:, 0:V], lgb[:], lsm[:, 0:1])
            nc.vector.tensor_scalar(
                ov[:, V:V + 1], lsm[:], -1.0, NEG,
                op0=ALU.mult, op1=ALU.add)
            # out rides the ACT queue: keeps the SP/Pool queues free so the
            # next repeat's loads aren't serialized behind this repeat's tail
            nc.scalar.dma_start(p_out[:], ov[:])

    nc.compile()
    return nc


def _fit_poly(X, n=NPOLY):
    """Least-squares odd-poly fit of tanh on [-X, X] at Chebyshev nodes."""
    x = X * np.cos(np.linspace(0, np.pi, 4001))
    pows = np.arange(1, n + 1, 2)
    M = x[:, None] ** pows[None, :]
    c, *_ = np.linalg.lstsq(M, np.tanh(x), rcond=None)
    cj = np.zeros(n + 1)
    cj[pows] = c
    return cj


def _prep(queries, keys, values, mask, Wc, bc, Wl, bl):
    """Host-side sharding: returns (V, in_maps, idx_valid, idx_masked)."""
    mask = np.asarray(mask)
    idx_v = [np.nonzero(mask[b])[0] for b in range(B)]
    idx_m = [np.nonzero(mask[b] == 0)[0] for b in range(B)]
    maxv = max(len(ix) for ix in idx_v)
    V = max(136, -(-maxv // 8) * 8)

    bf = ml_dtypes.bfloat16
    q_np = np.asarray(queries, np.float32)
    k_np = np.asarray(keys, np.float32)
    Wc_np = np.asarray(Wc, np.float32)
    bc_np = np.asarray(bc, np.float32)
    Wl_np = np.asarray(Wl, np.float32)[0]
    blv = float(np.asarray(bl, np.float32)[0])

    # poly fit on the exact realized range of A + K (computed on host; the
    # projections are cheap in fp32 BLAS and only run on the correctness path)
    qp = np.einsum('bhqd,ed->bqe', q_np, Wc_np[:, :D], optimize=True)
    kp = np.einsum('bhkd,ed->bke', k_np, Wc_np[:, D:], optimize=True) + bc_np
    xmax = max(float((qp[b].max(0) + kp[b].max(0)).max()) for b in range(B))
    xmin = min(float((qp[b].min(0) + kp[b].min(0)).min()) for b in range(B))
    X = max(abs(xmax), abs(xmin)) * 1.02
    cj = _fit_poly(X)
    coefs = np.asarray(_chain_coeffs(cj), np.float32)
    ncoef = len(coefs)

    wct_full = Wc_np.T.astype(bf)       # [2D, D]
    wct = np.empty((4, 128, 1024), bf)
    for ec in range(4):
        for dc in range(4):
            wct[ec, :, dc * 128:(dc + 1) * 128] = \
                wct_full[dc * 128:(dc + 1) * 128, ec * 128:(ec + 1) * 128]
            wct[ec, :, 512 + dc * 128:512 + (dc + 1) * 128] = \
                wct_full[D + dc * 128:D + (dc + 1) * 128, ec * 128:(ec + 1) * 128]
    bcp = bc_np.reshape(4, 128).T
    wlp = Wl_np.reshape(4, 128).T
    p0 = np.repeat(wlp.T.astype(bf)[:, :, None], 128, axis=2) \
        .transpose(1, 0, 2).reshape(128, 512)

    in_maps = []
    for c in range(NCORES):
        b, qh = c // 2, c % 2
        qt_d = q_np[b, 0, qh * LQL:(qh + 1) * LQL, :].T.astype(bf)   # [D, LQL]
        qt = qt_d.reshape(4, 128, LQL).transpose(1, 0, 2).reshape(128, 512)
        ktc = np.zeros((D, V), bf)
        ktc[:, :len(idx_v[b])] = k_np[b, 0, idx_v[b], :].T.astype(bf)
        kt = ktc.reshape(4, 128, V).transpose(1, 0, 2).reshape(128, 4 * V)
        qkp = np.concatenate([qt, p0, kt], axis=1)
        bvrow = np.full(V, NEG, np.float32)
        bvrow[:len(idx_v[b])] = 1.0 + blv
        aux = np.empty((128, 8 + ncoef + V), np.float32)
        aux[:, 0:4] = bcp
        aux[:, 4:8] = wlp
        aux[:, 8:8 + ncoef] = coefs[None, :]
        aux[:, 8 + ncoef:] = bvrow[None, :]
        in_maps.append({
            "qkp": np.ascontiguousarray(qkp), "wct": wct,
            "aux": np.ascontiguousarray(aux),
        })
    return V, in_maps, idx_v, idx_m


def kernel(queries, keys, values, mask, Wc, bc, Wl, bl):
    V, in_maps, idx_v, idx_m = _prep(queries, keys, values, mask, Wc, bc, Wl, bl)
    if V not in _nc_cache:
        _nc_cache[V] = _build(V)
    nc = _nc_cache[V]
    res = run_bass_kernel_spmd(nc, in_maps, core_ids=list(range(NCORES))).results

    full = np.empty((B, Lq, Lkv), np.float32)
    for c in range(NCORES):
        b, qh = c // 2, c % 2
        o = np.asarray(res[c]["out"], np.float32)      # [128, V+1]
        nv = len(idx_v[b])
        blk = full[b, qh * LQL:(qh + 1) * LQL]          # [128, Lkv]
        blk[:, idx_v[b]] = o[:, :nv]
        blk[:, idx_m[b]] = o[:, V:V + 1]
    return full


# revision 22
# speedup vs baseline: 59.5195x; 1.5821x over previous
"""Additive (Bahdanau) attention log-softmax weights on 8 TRN2 NeuronCores.

Math (per batch b, head 0):
    qp = Q @ Wq^T ; kp = K @ Wk^T          (Wc = [Wq | Wk], both [D, D])
    logit[q, k] = Wl . tanh(qp[q] + kp[k] + bc) + bl + where(mask[k]==0, -1e9, 1.0)
    out[q, :]   = log_softmax(logit[q, :])

Distribution: pure data parallel, core c <- (batch b = c//2, q-half c%2),
no collectives.  Sparse-attention trick: keys with mask==0 only need
out = -1e9 - LSE (error O(1) vs magnitude 1e9), so the device only computes
the ~136 valid keys (host compacts + pads to V).

Algorithm: polynomial separation instead of elementwise tanh.  With
A[e,q] = qp^T and K[e,k] = kp^T + bc, fit an odd minimax polynomial
p(x) = sum_j c_j x^j ~ tanh(x) on the exact realized range of A+K, then

    logit[q,k] ~ sum_e Wl[e] p(A+K) = sum_m <(Wl .* A^m)[:,q], R_m(K)[:,k]>

where R_m(K) = sum_l c_{m+l} C(m+l,m) K^l is elementwise in K.  The m=n
term is constant in k and cancels in log_softmax -> dropped.  This turns
8.9M tanh evaluations per core (58us on ACT at 1 col/cycle) into
 - a rank-(n*512) PE contraction (36 matmuls, ~4us),
 - Horner chains for R_m on DVE over only [512, V] elements (~12us),
 - powers A^m on DVE over [512, 128] (~3us),
and the logits land dense in PSUM [q, V], so softmax needs no gather.

Per-core layout (e' = partition, 4 e-chunks "ec" in the free dim):
  A, P_m : [128, 512]  col = ec*128 + q      (bf16)
  K, K2, R_m : [128, 4V]  col = ec*V + kc    (bf16)
  psL    : PSUM [128q, V] accumulating all 36 (m, ec) matmuls.
R_m chains are emitted step-round-robin (m descending) so R_8 finishes
first and PE starts accumulating while R_0 is still being built.

Timing note: the repeat-slope NEFF shares ONE output DRAM tensor across
repeats -- the axon tunnel charges ~80ms per output tensor (independent
of size/compute), which would otherwise dominate the slope.
"""

import numpy as np
import ml_dtypes
from contextlib import ExitStack

import concourse.bass as bass
import concourse.tile as tile
from concourse import bacc, mybir
from concourse.bass_utils import run_bass_kernel_spmd

F32 = mybir.dt.float32
BF16 = mybir.dt.bfloat16
AF = mybir.ActivationFunctionType
ALU = mybir.AluOpType

B, H, Lq, Lkv, D = 4, 1, 256, 256, 512
NCORES = 8
LQL = Lq // 2          # q rows per core
NEG = -1.0e9
NPOLY = 7              # odd minimax degree for tanh
ACT_HEAD_MIN = 4       # chains with m >= this run their head on ACT

_nc_cache: dict[int, object] = {}


def _chain_specs(n=None):
    """Per-m Horner chain structure for R_m(K) = sum_l beta_l K^l with
    m+l odd, l <= n-m.  Returns [(m, kfactor, d2)] with d2 = degree in K2
    of the inner polynomial; m even -> R_m = K * poly_d2(K2) (no constant),
    m odd -> R_m = poly_d2(K2) (with constant).  m = n dropped (k-constant
    term cancels in log_softmax)."""
    if n is None:
        n = NPOLY
    specs = []
    for m in range(n):
        lmax = n - m if (n - m + m) % 2 == 1 else n - m - 1
        # l of the same parity as (odd - m): l parity = (1 - m%2)
        if m % 2 == 0:
            d2 = (lmax - 1) // 2      # l = 1, 3, ..., lmax = 2*d2+1
            specs.append((m, True, d2))
        else:
            d2 = lmax // 2            # l = 0, 2, ..., lmax = 2*d2
            specs.append((m, False, d2))
    return specs


def _chain_coeffs(cj, n=None):
    """Flat coefficient list in the exact order _build consumes them.
    For chain m the Horner (over y=K2) coefficients are beta_{l(top)} ...
    beta_{l(bottom)}, where beta_l = c_{m+l} * C(m+l, m)."""
    from math import comb
    if n is None:
        n = NPOLY
    out = []
    for m, kfac, d2 in _chain_specs(n):
        if kfac:
            ls = [2 * i + 1 for i in range(d2, -1, -1)]
        else:
            ls = [2 * i for i in range(d2, -1, -1)]
        for l in ls:
            out.append(float(cj[m + l]) * comb(m + l, m))
    return out


def _build(V: int, repeats: int = 1, skip=(), pchain_dve=True):
    """Build + schedule the per-core Bass graph for padded-valid-count V.

    skip / pchain_dve are timing-ablation knobs (results become garbage for
    most of them); the real kernel uses the defaults."""
    nc = bacc.Bacc(None, target_bir_lowering=False)

    specs = _chain_specs()
    ncoef = sum(d2 + 1 for _, _, d2 in specs)

    # packed inputs: qkp = [qt | p0 | kt] bf16, aux = [bcp | wlp | coef | bv]
    # f32 -- each dma_start costs ~0.6us of sequencer dispatch, so small
    # tensors ride together.  wct loads go on the ACT queue so they overlap
    # the qkp/aux loads on the SP queue.
    p_qkp = nc.declare_dram_parameter("qkp", [128, 1024 + 4 * V], BF16,
                                      isOutput=False)
    p_wct = nc.declare_dram_parameter("wct", [4, 128, 1024], BF16, isOutput=False)
    p_aux = nc.declare_dram_parameter("aux", [128, 8 + ncoef + V], F32,
                                      isOutput=False)
    # One output tensor shared by every repeat: the axon tunnel charges a
    # large fixed cost PER OUTPUT TENSOR, so the timing NEFF must not scale
    # its output count with R (WAW between repeats is queue-ordered).
    p_out = nc.declare_dram_parameter("out", [128, V + 1], F32, isOutput=True)

    with ExitStack() as ctx:
        tc = ctx.enter_context(tile.TileContext(nc))
        const = ctx.enter_context(tc.tile_pool(name="const", bufs=1))
        psum = ctx.enter_context(tc.tile_pool(name="psum", bufs=2, space="PSUM"))

        for rep in range(repeats):
            r = f"_r{rep}"
            # ---- loads (wct on the ACT queue, rest on SP queue) ----
            wct_t = [const.tile([128, 1024], BF16, tag=f"wct{ec}",
                                name=f"wct{ec}{r}") for ec in range(4)]
            if "loads" not in skip:
                for ec in range(4):
                    nc.gpsimd.dma_start(wct_t[ec][:], p_wct[ec])
            qkp_t = const.tile([128, 1024 + 4 * V], BF16, tag="qkp",
                               name=f"qkp{r}")
            if "loads" not in skip:
                nc.sync.dma_start(qkp_t[:], p_qkp[:])
            aux_t = const.tile([128, 8 + ncoef + V], F32, tag="aux",
                               name=f"aux{r}")
            if "loads" not in skip:
                nc.sync.dma_start(aux_t[:], p_aux[:])
            qt_t = qkp_t[:, 0:512]
            p0_t = qkp_t[:, 512:1024]
            kt_t = qkp_t[:, 1024:1024 + 4 * V]
            bcp_t = aux_t[:, 0:4]
            wlp_t = aux_t[:, 4:8]
            coef_t = aux_t[:, 8:8 + ncoef]
            bv_t = aux_t[:, 8 + ncoef:8 + ncoef + V]

            # ---- projections: qp^T -> A/P1, kp^T + bc -> K (folds on the
            #      otherwise-idle ACT engine; per-partition bias/scale APs) --
            K = const.tile([128, 4 * V], BF16, tag="K", name=f"K{r}")
            K2 = const.tile([128, 4 * V], BF16, tag="K2", name=f"K2{r}")
            A = const.tile([128, 512], BF16, tag="A", name=f"A{r}")
            P = [p0_t] + [const.tile([128, 512], BF16, tag=f"P{m}",
                                     name=f"P{m}{r}") for m in range(1, NPOLY)]
            R = [const.tile([128, 4 * V], BF16, tag=f"R{m}", name=f"R{m}{r}")
                 for m in range(NPOLY)]
            # k-projections first: the K -> K2 -> R chains are the critical
            # path; q-projections only feed the P side which has slack
            if "proj" in skip:
                nc.gpsimd.memset(K[:], 0.001)
                nc.gpsimd.memset(A[:], 0.001)
                nc.gpsimd.memset(P[1][:], 0.001)
            if "k2" in skip:
                nc.gpsimd.memset(K2[:], 0.001)
            if "loads" in skip:
                nc.gpsimd.memset(qkp_t[:], 0.001)
                nc.gpsimd.memset(aux_t[:], 0.001)
                for ec in range(4):
                    nc.gpsimd.memset(wct_t[ec][:], 0.001)
            if "proj" not in skip:
                for ec in range(4):
                    ps_k = psum.tile([128, V], F32, tag="psk", name=f"psk{ec}{r}")
                    for dc in range(4):
                        nc.tensor.matmul(
                            ps_k[:],
                            wct_t[ec][:, 512 + dc * 128:512 + (dc + 1) * 128],
                            kt_t[:, dc * V:(dc + 1) * V],
                            start=(dc == 0), stop=(dc == 3))
                    nc.scalar.activation(
                        K[:, ec * V:(ec + 1) * V], ps_k[:], AF.Identity,
                        bias=bcp_t[:, ec:ec + 1])
            if "k2" not in skip:
                nc.scalar.activation(K2[:], K[:], AF.Square)
            if "proj" not in skip:
                for ec in range(4):
                    ps_q = psum.tile([128, 128], F32, tag="psq", name=f"psq{ec}{r}")
                    for dc in range(4):
                        nc.tensor.matmul(
                            ps_q[:], wct_t[ec][:, dc * 128:(dc + 1) * 128],
                            qt_t[:, dc * 128:(dc + 1) * 128],
                            start=(dc == 0), stop=(dc == 3))
                    nc.scalar.activation(
                        A[:, ec * 128:(ec + 1) * 128], ps_q[:], AF.Copy)
                    nc.scalar.activation(
                        P[1][:, ec * 128:(ec + 1) * 128], ps_q[:], AF.Copy,
                        scale=wlp_t[:, ec:ec + 1])

            # R_m Horner chains over K2.  Chain heads (beta_top*K2 +
            # beta_next, a pure scale/bias op) for m >= ACT_HEAD_MIN run as
            # ACT Identity/Copy instructions with per-partition scale+bias
            # APs, balancing the two engines; the tensor*tensor inner mads
            # must stay on DVE.  DVE stream order: heads (m desc), then the
            # P-power chain (A is ready by then), then the inner mads
            # round-robin so high-m chains finish first for PE.
            cbase = {}
            idx = 0
            for m, kfac, d2 in specs:
                cbase[m] = idx
                idx += d2 + 1
            order = [s for s in reversed(specs)]   # m desc
            maxsteps = max(d2 for _, _, d2 in specs) + 2
            done = set()
            if "rchain" in skip:
                for m in range(NPOLY):
                    nc.gpsimd.memset(R[m][:], 0.001)

            def emit_head(m, kfac, d2):
                acc, c0 = R[m], cbase[m]
                if m >= ACT_HEAD_MIN:
                    if d2 == 0:
                        assert kfac
                        nc.scalar.activation(acc[:], K[:], AF.Copy,
                                             scale=coef_t[:, c0:c0 + 1])
                        done.add(m)
                    else:
                        nc.scalar.activation(acc[:], K2[:], AF.Identity,
                                             bias=coef_t[:, c0 + 1:c0 + 2],
                                             scale=coef_t[:, c0:c0 + 1])
                elif d2 == 0:
                    assert kfac
                    nc.vector.tensor_scalar_mul(acc[:], K[:],
                                                coef_t[:, c0:c0 + 1])
                    done.add(m)
                else:
                    # fused (K2 * beta_top + beta_next) in one 4x instr
                    nc.vector.tensor_scalar(
                        acc[:], K2[:], coef_t[:, c0:c0 + 1],
                        coef_t[:, c0 + 1:c0 + 2], op0=ALU.mult, op1=ALU.add)

            if "rchain" not in skip:
                for m, kfac, d2 in order:
                    emit_head(m, kfac, d2)

            if "pchain" in skip:
                for m in range(2, NPOLY):
                    nc.gpsimd.memset(P[m][:], 0.001)
            else:
                eng = nc.vector if pchain_dve else nc.gpsimd
                for m in range(2, NPOLY):
                    eng.tensor_tensor(P[m][:], P[m - 1][:], A[:], ALU.mult)

            for step in range(1 if "rchain" not in skip else 10**9,
                              maxsteps + 1):
                for m, kfac, d2 in order:
                    if m in done:
                        continue
                    acc = R[m]
                    c0 = cbase[m]
                    if step < d2:
                        nc.vector.tensor_tensor(acc[:], acc[:], K2[:], ALU.mult)
                        nc.vector.tensor_scalar_add(
                            acc[:], acc[:], coef_t[:, c0 + step + 1:c0 + step + 2])
                    elif step == d2:
                        if kfac:
                            nc.vector.tensor_tensor(acc[:], acc[:], K[:], ALU.mult)
                        done.add(m)
            # ---- PE: logits = sum_m <P_m, R_m>, m descending ----
            psL = psum.tile([128, V], F32, tag="psL", name=f"psL{r}")
            mm = [(m, ec) for m in range(NPOLY - 1, -1, -1) for ec in range(4)]
            if "mm" in skip:
                mm = mm[:1]
            for i, (m, ec) in enumerate(mm):
                nc.tensor.matmul(
                    psL[:], P[m][:, ec * 128:(ec + 1) * 128],
                    R[m][:, ec * V:(ec + 1) * V],
                    start=(i == 0), stop=(i == len(mm) - 1))

            # ---- softmax tail ----
            lgb = const.tile([128, V], F32, tag="lgb", name=f"lgb{r}")
            ex = const.tile([128, V], F32, tag="ex", name=f"ex{r}")
            sm = const.tile([128, 1], F32, tag="sm", name=f"sm{r}")
            lsm = const.tile([128, 1], F32, tag="lsm", name=f"lsm{r}")
            ov = const.tile([128, V + 1], F32, tag="ov", name=f"ov{r}")
            nc.vector.tensor_tensor(lgb[:], psL[:], bv_t[:], ALU.add)
            nc.scalar.activation(ex[:], lgb[:], AF.Exp, accum_out=sm[:])
            nc.scalar.activation(lsm[:], sm[:], AF.Ln)
            nc.vector.tensor_scalar_sub(ov[:, 0:V], lgb[:], lsm[:, 0:1])
            nc.vector.tensor_scalar(
                ov[:, V:V + 1], lsm[:], -1.0, NEG,
                op0=ALU.mult, op1=ALU.add)
            # out rides the ACT queue: keeps the SP/Pool queues free so the
            # next repeat's loads aren't serialized behind this repeat's tail
            nc.scalar.dma_start(p_out[:], ov[:])

    nc.compile()
    return nc


def _fit_poly(X, n=NPOLY):
    """Least-squares odd-poly fit of tanh on [-X, X] at Chebyshev nodes."""
    x = X * np.cos(np.linspace(0, np.pi, 4001))
    pows = np.arange(1, n + 1, 2)
    M = x[:, None] ** pows[None, :]
    c, *_ = np.linalg.lstsq(M, np.tanh(x), rcond=None)
    cj = np.zeros(n + 1)
    cj[pows] = c
    return cj


def _prep(queries, keys, values, mask, Wc, bc, Wl, bl):
    """Host-side sharding: returns (V, in_maps, idx_valid, idx_masked)."""
    mask = np.asarray(mask)
    idx_v = [np.nonzero(mask[b])[0] for b in range(B)]
    idx_m = [np.nonzero(mask[b] == 0)[0] for b in range(B)]
    maxv = max(len(ix) for ix in idx_v)
    V = max(136, -(-maxv // 8) * 8)

    bf = ml_dtypes.bfloat16
    q_np = np.asarray(queries, np.float32)
    k_np = np.asarray(keys, np.float32)
    Wc_np = np.asarray(Wc, np.float32)
    bc_np = np.asarray(bc, np.float32)
    Wl_np = np.asarray(Wl, np.float32)[0]
    blv = float(np.asarray(bl, np.float32)[0])

    # poly fit on the exact realized range of A + K (computed on host; the
    # projections are cheap in fp32 BLAS and only run on the correctness path)
    qp = np.einsum('bhqd,ed->bqe', q_np, Wc_np[:, :D], optimize=True)
    kp = np.einsum('bhkd,ed->bke', k_np, Wc_np[:, D:], optimize=True) + bc_np
    xmax = max(float((qp[b].max(0) + kp[b].max(0)).max()) for b in range(B))
    xmin = min(float((qp[b].min(0) + kp[b].min(0)).min()) for b in range(B))
    X = max(abs(xmax), abs(xmin)) * 1.02
    cj = _fit_poly(X)
    coefs = np.asarray(_chain_coeffs(cj), np.float32)
    ncoef = len(coefs)

    wct_full = Wc_np.T.astype(bf)       # [2D, D]
    wct = np.empty((4, 128, 1024), bf)
    for ec in range(4):
        for dc in range(4):
            wct[ec, :, dc * 128:(dc + 1) * 128] = \
                wct_full[dc * 128:(dc + 1) * 128, ec * 128:(ec + 1) * 128]
            wct[ec, :, 512 + dc * 128:512 + (dc + 1) * 128] = \
                wct_full[D + dc * 128:D + (dc + 1) * 128, ec * 128:(ec + 1) * 128]
    bcp = bc_np.reshape(4, 128).T
    wlp = Wl_np.reshape(4, 128).T
    p0 = np.repeat(wlp.T.astype(bf)[:, :, None], 128, axis=2) \
        .transpose(1, 0, 2).reshape(128, 512)

    in_maps = []
    for c in range(NCORES):
        b, qh = c // 2, c % 2
        qt_d = q_np[b, 0, qh * LQL:(qh + 1) * LQL, :].T.astype(bf)   # [D, LQL]
        qt = qt_d.reshape(4, 128, LQL).transpose(1, 0, 2).reshape(128, 512)
        ktc = np.zeros((D, V), bf)
        ktc[:, :len(idx_v[b])] = k_np[b, 0, idx_v[b], :].T.astype(bf)
        kt = ktc.reshape(4, 128, V).transpose(1, 0, 2).reshape(128, 4 * V)
        qkp = np.concatenate([qt, p0, kt], axis=1)
        bvrow = np.full(V, NEG, np.float32)
        bvrow[:len(idx_v[b])] = 1.0 + blv
        aux = np.empty((128, 8 + ncoef + V), np.float32)
        aux[:, 0:4] = bcp
        aux[:, 4:8] = wlp
        aux[:, 8:8 + ncoef] = coefs[None, :]
        aux[:, 8 + ncoef:] = bvrow[None, :]
        in_maps.append({
            "qkp": np.ascontiguousarray(qkp), "wct": wct,
            "aux": np.ascontiguousarray(aux),
        })
    return V, in_maps, idx_v, idx_m


def kernel(queries, keys, values, mask, Wc, bc, Wl, bl):
    V, in_maps, idx_v, idx_m = _prep(queries, keys, values, mask, Wc, bc, Wl, bl)
    if V not in _nc_cache:
        _nc_cache[V] = _build(V)
    nc = _nc_cache[V]
    res = run_bass_kernel_spmd(nc, in_maps, core_ids=list(range(NCORES))).results

    full = np.empty((B, Lq, Lkv), np.float32)
    for c in range(NCORES):
        b, qh = c // 2, c % 2
        o = np.asarray(res[c]["out"], np.float32)      # [128, V+1]
        nv = len(idx_v[b])
        blk = full[b, qh * LQL:(qh + 1) * LQL]          # [128, Lkv]
        blk[:, idx_v[b]] = o[:, :nv]
        blk[:, idx_m[b]] = o[:, V:V + 1]
    return full
